# revision 49
# baseline (speedup 1.0000x reference)
"""Self-contained Trainium2 Bass kernel for nn_Attention (B=8, N=1024, C=768, NH=12).

Sharding: pure data-parallel over batch — core b computes batch element b
(projections, 12-head biased softmax attention, attn output, out projection).

Device algorithm per core (matmul inputs bf16, fp32 PSUM accumulate):
  phase 1: qT[e,n], kT[e,n] (e = head*64+d on partitions) and v[m,dd] from
           host-pretransposed xT and weights. 1/sqrt(d) folded into Wq on host.
  phase 2 per head h:
    A-side ([n on partitions]): scores = qT.T@kT + bias (bias fp8, added via
      identity-matmul into the same PSUM accumulation), one Exp on ScalarE
      with fused row-sum accum, per-partition reciprocal, normalize on
      VectorE -> attn tile (bf16) -> DMA to HBM.
    B-side: one DMA-transpose read per (pair, m-block) brings attn of BOTH
      heads back column-wise (bf16 X-bar, [2N,128]->[128,2N]); the two heads'
      U^T = v_h.T @ attnT matmuls accumulate into shared PSUM banks via
      disjoint column groups (concurrent on the PE array), evicted pair-wide
      to outT (c on partitions).
  phase 3: y = outT.T @ Wproj + proj_b, DMA out.

attn is produced in bf16 on device and upcast to f32 on the host.
proj_b is added on the PE (identity-matmul, bf16) and y evicted on ScalarE.

Cost-model (TimelineSim) exec: ~298 us/core; measured rel_err ~2.8e-3.
"""

from contextlib import ExitStack

import numpy as np
import ml_dtypes

import concourse.mybir as mybir
import concourse.tile as tile
from concourse import bacc
from concourse.bass_utils import run_bass_kernel_spmd
from concourse.masks import make_identity
from concourse.tile_rust import add_dep_helper

BF16 = mybir.dt.bfloat16
FP8 = mybir.dt.float8e4
F32 = mybir.dt.float32
NP_BF16 = ml_dtypes.bfloat16
NP_FP8 = ml_dtypes.float8_e4m3

B, N, C, NH = 8, 1024, 768, 12
D = C // NH  # 64
CT = C // 128  # 6 contraction tiles
NT = N // 128  # 8 sequence tiles
Exp = mybir.ActivationFunctionType.Exp

MARKS = []


def build_graph():
    MARKS.clear()
    nc = bacc.Bacc("TRN2", target_bir_lowering=False, debug=False)
    mark = lambda label: MARKS.append((label, nc.next_id()))

    xt_d = nc.declare_dram_parameter("xt", [C, N], BF16, False).ap()
    wq_d = nc.declare_dram_parameter("wq", [C, C], BF16, False).ap()
    wk_d = nc.declare_dram_parameter("wk", [C, C], BF16, False).ap()
    wv_d = nc.declare_dram_parameter("wv", [C, C], BF16, False).ap()
    wp_d = nc.declare_dram_parameter("wp", [C, C], BF16, False).ap()
    pb_d = nc.declare_dram_parameter("pb", [128, C], BF16, False).ap()
    biasa_d = nc.declare_dram_parameter("biasa", [NH, N, N], FP8, False).ap()
    out_d = nc.declare_dram_parameter("out", [N, C], F32, True).ap()
    attn_d = nc.declare_dram_parameter("attn", [NH, N, N], BF16, True).ap()

    with ExitStack() as ctx:
        tc = ctx.enter_context(tile.TileContext(nc))
        const = ctx.enter_context(tc.tile_pool(name="const", bufs=1))
        persist = ctx.enter_context(tc.tile_pool(name="persist", bufs=1))
        biasp = ctx.enter_context(tc.tile_pool(name="biasp", bufs=8))
        expap = ctx.enter_context(tc.tile_pool(name="expap", bufs=7))
        exptp = ctx.enter_context(tc.tile_pool(name="exptp", bufs=5))
        stage = ctx.enter_context(tc.tile_pool(name="stage", bufs=10))
        stats = ctx.enter_context(tc.tile_pool(name="stats", bufs=4))
        psmm = ctx.enter_context(tc.tile_pool(name="psmm", bufs=3, space="PSUM"))
        psu = ctx.enter_context(tc.tile_pool(name="psu", bufs=2, space="PSUM"))

        ident8 = const.tile([128, 128], FP8, name="ident8")
        make_identity(nc, ident8[:])
        identb = const.tile([128, 128], BF16, name="identb")
        make_identity(nc, identb[:])

        xt_sb = persist.tile([128, CT, N], BF16, name="xt_sb")
        xt_r = xt_d.rearrange("(t p) n -> p t n", p=128)
        for ct in range(CT):
            nc.sync.dma_start(xt_sb[:, ct, :], xt_r[:, ct, :])
        w_sb = {}
        for nm, d_ap in (("wq", wq_d), ("wk", wk_d), ("wv", wv_d), ("wp", wp_d)):
            w_sb[nm] = persist.tile([128, CT, C], BF16, name=f"{nm}_sb")
            nc.sync.dma_start(w_sb[nm][:], d_ap.rearrange("(t p) e -> p t e", p=128))
        pb_sb = persist.tile([128, C], BF16, name="pb_sb")
        nc.sync.dma_start(pb_sb[:], pb_d)

        qT = persist.tile([128, CT, N], BF16, name="qT")
        kT = persist.tile([128, CT, N], BF16, name="kT")
        vX = persist.tile([128, NT, C], BF16, name="vX")
        outT = persist.tile([128, CT, N], BF16, name="outT")

        mark("phase1")
        # ---- phase 1: projections ----
        # chunk order (0,0),(q/k interleaved) first so the head-0/1 attention
        # pipeline (and its DMA traffic) can start while phase 1 continues.
        qk_order = [("wq", qT, 0), ("wk", kT, 0)]
        for e in range(1, CT):
            qk_order += [("wq", qT, e), ("wk", kT, e)]
        for wname, dst, e in qk_order:
            if True:
                for nh2 in range(2):
                    ps = psmm.tile([128, 512], F32, tag="mm", name="ps_qk")
                    for ct in range(CT):
                        nc.tensor.matmul(
                            ps[:],
                            w_sb[wname][:, ct, 128 * e : 128 * (e + 1)],
                            xt_sb[:, ct, 512 * nh2 : 512 * (nh2 + 1)],
                            start=(ct == 0),
                            stop=(ct == CT - 1),
                        )
                    nc.vector.tensor_copy(
                        dst[:, e, 512 * nh2 : 512 * (nh2 + 1)], ps[:]
                    )
        for c0, cw in ((0, 512), (512, 256)):
            for mt in range(NT):
                ps = psmm.tile([128, 512], F32, tag="mm", name="ps_v")
                for ct in range(CT):
                    nc.tensor.matmul(
                        ps[:, 0:cw],
                        xt_sb[:, ct, 128 * mt : 128 * (mt + 1)],
                        w_sb["wv"][:, ct, c0 : c0 + cw],
                        start=(ct == 0),
                        stop=(ct == CT - 1),
                    )
                nc.vector.tensor_copy(vX[:, mt, c0 : c0 + cw], ps[:, 0:cw])

        # ---- phase 2: attention, head pairs ----
        # Heads 2j (partitions 0:64) and 2j+1 (64:128) of qT/kT chunk j are
        # processed together: their K=64 scores matmuls are emitted adjacently
        # into different PE row groups, which the hardware runs concurrently.
        # Bias tiles are DMA'd one pair ahead to hide the load latency.
        ba_tiles = {}

        def prefetch_bias(h):
            if h >= NH:
                return
            tiles = []
            for half in range(2):
                ba = biasp.tile([128, 4, N], FP8, tag="biasa", name="ba")
                nc.sync.dma_start(
                    ba[:],
                    biasa_d[h, 512 * half : 512 * (half + 1), :].rearrange(
                        "(t p) m -> p t m", p=128
                    ),
                )
                tiles.append(ba)
            ba_tiles[h] = tiles

        attn_wrs = {}

        def emit_A_pair(j):
            ha, hb = 2 * j, 2 * j + 1
            mark(f"p{j}_A")
            attn_wrs[ha] = []
            attn_wrs[hb] = []
            for half in range(2):
                for nt4 in range(4):
                    nt = 4 * half + nt4
                    pss = {}
                    for h in (ha, hb):
                        pss[h] = psmm.tile([128, N], F32, tag="mm", name="ps_sa")
                    for mh in range(2):
                        for h in (ha, hb):
                            po = (h % 2) * 64
                            nc.tensor.matmul(
                                pss[h][:, 512 * mh : 512 * (mh + 1)],
                                qT[po : po + 64, j, 128 * nt : 128 * (nt + 1)],
                                kT[po : po + 64, j, 512 * mh : 512 * (mh + 1)],
                                start=True,
                                stop=False,
                            )
                        for h in (ha, hb):
                            nc.tensor.matmul(
                                pss[h][:, 512 * mh : 512 * (mh + 1)],
                                ident8[:],
                                ba_tiles[h][half][:, nt4, 512 * mh : 512 * (mh + 1)],
                                start=False,
                                stop=True,
                            )
                    for h in (ha, hb):
                        ex = expap.tile([128, N], BF16, tag="expA", name="ex")
                        ssum = stats.tile([128, 1], F32, tag="ssum", name="ssum")
                        nc.scalar.activation(ex[:], pss[h][:], Exp, accum_out=ssum[:])
                        recip = stats.tile([128, 1], F32, tag="recip", name="recip")
                        nc.vector.reciprocal(recip[:], ssum[:])
                        ast = stage.tile([128, N], BF16, tag="attnstage", name="ast")
                        nc.vector.tensor_scalar_mul(ast[:], ex[:], recip[:])
                        wr = nc.sync.dma_start(
                            attn_d[h, 128 * nt : 128 * (nt + 1), :], ast[:]
                        )
                        attn_wrs[h].append(wr)

        def emit_B_pair(j):
            # One transpose read per (pair, m-block): rows (h*N + n) for both
            # heads -> attnT [m, 2N]. The two heads' U matmuls share PSUM
            # banks via disjoint column groups (0:64 / 64:128).
            ha, hb = 2 * j, 2 * j + 1
            mark(f"p{j}_B")
            psu_t = []
            for nh in range(2):
                psu_t.append(psu.tile([128, 512], F32, tag="u", name="psu_t"))
            for mt in range(NT):
                at = exptp.tile([128, 2 * N], BF16, tag="attnT", name="at")
                rd = nc.sync.dma_start(
                    at[:],
                    attn_d[ha : hb + 1, :, 128 * mt : 128 * (mt + 1)].rearrange(
                        "h n m -> (h n) m"
                    ),
                    transpose=True,
                )
                for wr in attn_wrs[ha] + attn_wrs[hb]:
                    add_dep_helper(rd.ins, wr.ins, reason="attn HBM round-trip RAW")
                for nh in range(2):
                    for idx, h in enumerate((ha, hb)):
                        po = (h % 2) * 64
                        nc.tensor.matmul(
                            psu_t[nh][po : po + 64, :],
                            vX[:, mt, 64 * h : 64 * (h + 1)],
                            at[:, 1024 * idx + 512 * nh : 1024 * idx + 512 * (nh + 1)],
                            start=(mt == 0),
                            stop=(mt == NT - 1),
                        )
            mark(f"p{j}_tail")
            for nh in range(2):
                nc.vector.tensor_copy(
                    outT[:, j, 512 * nh : 512 * (nh + 1)], psu_t[nh][:]
                )

        for h0 in range(4):
            prefetch_bias(h0)
        for j in range(NH // 2):
            emit_A_pair(j)
            prefetch_bias(2 * j + 4)
            prefetch_bias(2 * j + 5)
            emit_B_pair(j)

        mark("phase3")
        # ---- phase 3: output projection ----
        for nt in range(NT):
            ysb = stage.tile([128, C], F32, tag="y", name="ysb", bufs=2)
            for c0, cw in ((0, 512), (512, 256)):
                ps = psmm.tile([128, 512], F32, tag="mm", name="ps_y")
                for ct in range(CT):
                    nc.tensor.matmul(
                        ps[:, 0:cw],
                        outT[:, ct, 128 * nt : 128 * (nt + 1)],
                        w_sb["wp"][:, ct, c0 : c0 + cw],
                        start=(ct == 0),
                        stop=False,
                    )
                nc.tensor.matmul(
                    ps[:, 0:cw],
                    identb[:],
                    pb_sb[:, c0 : c0 + cw],
                    start=False,
                    stop=True,
                )
                nc.scalar.activation(
                    ysb[:, c0 : c0 + cw],
                    ps[:, 0:cw],
                    mybir.ActivationFunctionType.Copy,
                )
            nc.sync.dma_start(out_d[128 * nt : 128 * (nt + 1), :], ysb[:])

    return nc


_GRAPH_CACHE = {}


def _get_graph():
    if "nc" not in _GRAPH_CACHE:
        nc = build_graph()
        nc.compile()
        _GRAPH_CACHE["nc"] = nc
    return _GRAPH_CACHE["nc"]


def _prep_in_maps(x, Wq, Wk, Wv, Wproj, proj_b, attn_bias, head_bias):
    f = lambda a: np.asarray(a, dtype=np.float32)
    x, Wq, Wk, Wv, Wproj = f(x), f(Wq), f(Wk), f(Wv), f(Wproj)
    proj_b, attn_bias, head_bias = f(proj_b), f(attn_bias), f(head_bias)

    scale = D ** -0.5
    wq_b = (Wq * scale).astype(NP_BF16)
    wk_b = Wk.astype(NP_BF16)
    wv_b = Wv.astype(NP_BF16)
    wp_b = Wproj.astype(NP_BF16)
    bias = attn_bias[None, :, :] + head_bias  # [NH, N, N] f32
    biasa = bias.astype(NP_FP8)
    pb_rep = np.ascontiguousarray(
        np.broadcast_to(proj_b[None, :], (128, C))
    ).astype(NP_BF16)

    in_maps = []
    for b in range(B):
        xt = np.ascontiguousarray(x[b].T).astype(NP_BF16)
        in_maps.append(
            dict(xt=xt, wq=wq_b, wk=wk_b, wv=wv_b, wp=wp_b, pb=pb_rep, biasa=biasa)
        )
    return in_maps


def run(inputs: dict, trace: bool = False, **kw):
    """Build+run on 8 cores; returns (out, attn, BassKernelResults)."""
    in_maps = _prep_in_maps(**inputs)
    nc = _get_graph()
    res = run_bass_kernel_spmd(nc, in_maps, list(range(B)), trace=trace, **kw)
    out = np.stack([r["out"] for r in res.results]).astype(np.float32)
    attn = np.stack([r["attn"] for r in res.results]).astype(np.float32)
    return out, attn, res


def kernel(**inputs):
    out, attn, _ = run(inputs, trace=False)
    return out, attn


# revision 62
# speedup vs baseline: 1.0964x; 1.0964x over previous
"""Self-contained Trainium2 Bass kernel for nn_Attention (B=8, N=1024, C=768, NH=12).

Sharding: pure data-parallel over batch — core b computes batch element b
(projections, 12-head biased softmax attention, attn output, out projection).

Device algorithm per core (matmul inputs bf16, fp32 PSUM accumulate):
  phase 1: qT[e,n], kT[e,n] (e = head*64+d on partitions) and v[m,dd] from
           host-pretransposed xT and weights. 1/sqrt(d) folded into Wq on host.
  phase 2 per head h:
    A-side ([n on partitions]): scores = qT.T@kT + bias (bias fp8, added via
      identity-matmul into the same PSUM accumulation), one Exp on ScalarE
      with fused row-sum accum, per-partition reciprocal, normalize on
      VectorE -> attn tile (bf16) -> DMA to HBM.
    B-side: one DMA-transpose read per (pair, m-block) brings attn of BOTH
      heads back column-wise (bf16 X-bar, [2N,128]->[128,2N]); the two heads'
      U^T = v_h.T @ attnT matmuls accumulate into shared PSUM banks via
      disjoint column groups (concurrent on the PE array), evicted pair-wide
      to outT (c on partitions).
  phase 3: y = outT.T @ Wproj + proj_b, DMA out.

attn is produced in bf16 on device and upcast to f32 on the host.
proj_b is added on the PE (identity-matmul, bf16) and y evicted on ScalarE.

Cost-model (TimelineSim) exec: ~298 us/core; measured rel_err ~2.8e-3.
"""

from contextlib import ExitStack

import numpy as np
import ml_dtypes

import concourse.mybir as mybir
import concourse.tile as tile
from concourse import bacc
from concourse.bass_utils import run_bass_kernel_spmd
from concourse.masks import make_identity
from concourse.tile_rust import add_dep_helper

BF16 = mybir.dt.bfloat16
FP8 = mybir.dt.float8e4
F32 = mybir.dt.float32
NP_BF16 = ml_dtypes.bfloat16
NP_FP8 = ml_dtypes.float8_e4m3

B, N, C, NH = 8, 1024, 768, 12
D = C // NH  # 64
CT = C // 128  # 6 contraction tiles
NT = N // 128  # 8 sequence tiles
Exp = mybir.ActivationFunctionType.Exp

MARKS = []


def build_graph():
    MARKS.clear()
    nc = bacc.Bacc("TRN2", target_bir_lowering=False, debug=False)
    mark = lambda label: MARKS.append((label, nc.next_id()))

    xt_d = nc.declare_dram_parameter("xt", [C, N], BF16, False).ap()
    wq_d = nc.declare_dram_parameter("wq", [C, C], BF16, False).ap()
    wk_d = nc.declare_dram_parameter("wk", [C, C], BF16, False).ap()
    wv_d = nc.declare_dram_parameter("wv", [C, C], BF16, False).ap()
    wp_d = nc.declare_dram_parameter("wp", [C, C], BF16, False).ap()
    pb_d = nc.declare_dram_parameter("pb", [128, C], BF16, False).ap()
    biasa_d = nc.declare_dram_parameter("biasa", [NH, N, N], FP8, False).ap()
    out_d = nc.declare_dram_parameter("out", [N, C], F32, True).ap()
    attn_d = nc.declare_dram_parameter("attn", [NH, N, N], BF16, True).ap()

    with ExitStack() as ctx:
        tc = ctx.enter_context(tile.TileContext(nc))
        const = ctx.enter_context(tc.tile_pool(name="const", bufs=1))
        persist = ctx.enter_context(tc.tile_pool(name="persist", bufs=1))
        biasp = ctx.enter_context(tc.tile_pool(name="biasp", bufs=8))
        expap = ctx.enter_context(tc.tile_pool(name="expap", bufs=7))
        exptp = ctx.enter_context(tc.tile_pool(name="exptp", bufs=5))
        stage = ctx.enter_context(tc.tile_pool(name="stage", bufs=10))
        stats = ctx.enter_context(tc.tile_pool(name="stats", bufs=4))
        psmm = ctx.enter_context(tc.tile_pool(name="psmm", bufs=3, space="PSUM"))
        psu = ctx.enter_context(tc.tile_pool(name="psu", bufs=2, space="PSUM"))

        ident8 = const.tile([128, 128], FP8, name="ident8")
        make_identity(nc, ident8[:])
        identb = const.tile([128, 128], BF16, name="identb")
        make_identity(nc, identb[:])

        xt_sb = persist.tile([128, CT, N], BF16, name="xt_sb")
        xt_r = xt_d.rearrange("(t p) n -> p t n", p=128)
        for ct in range(CT):
            nc.sync.dma_start(xt_sb[:, ct, :], xt_r[:, ct, :])
        w_sb = {}
        for nm, d_ap in (("wq", wq_d), ("wk", wk_d), ("wv", wv_d), ("wp", wp_d)):
            w_sb[nm] = persist.tile([128, CT, C], BF16, name=f"{nm}_sb")
            nc.sync.dma_start(w_sb[nm][:], d_ap.rearrange("(t p) e -> p t e", p=128))
        pb_sb = persist.tile([128, C], BF16, name="pb_sb")
        nc.sync.dma_start(pb_sb[:], pb_d)

        qT = persist.tile([128, CT, N], BF16, name="qT")
        kT = persist.tile([128, CT, N], BF16, name="kT")
        vX = persist.tile([128, NT, C], BF16, name="vX")
        outT = persist.tile([128, CT, N], BF16, name="outT")

        mark("phase1")

        # ---- projections, emitted per head-pair chunk (see driver) ----
        def emit_qk_chunk(e):
            for wname, dst in (("wq", qT), ("wk", kT)):
                for nh2 in range(2):
                    ps = psmm.tile([128, 512], F32, tag="mm", name="ps_qk")
                    for ct in range(CT):
                        nc.tensor.matmul(
                            ps[:],
                            w_sb[wname][:, ct, 128 * e : 128 * (e + 1)],
                            xt_sb[:, ct, 512 * nh2 : 512 * (nh2 + 1)],
                            start=(ct == 0),
                            stop=(ct == CT - 1),
                        )
                    nc.vector.tensor_copy(
                        dst[:, e, 512 * nh2 : 512 * (nh2 + 1)], ps[:]
                    )

        def emit_v(c0, cw):
            mark(f"v{c0}")
            if True:
                for mt in range(NT):
                    ps = psmm.tile([128, 512], F32, tag="mm", name="ps_v")
                    for ct in range(CT):
                        nc.tensor.matmul(
                            ps[:, 0:cw],
                            xt_sb[:, ct, 128 * mt : 128 * (mt + 1)],
                            w_sb["wv"][:, ct, c0 : c0 + cw],
                            start=(ct == 0),
                            stop=(ct == CT - 1),
                        )
                    nc.vector.tensor_copy(vX[:, mt, c0 : c0 + cw], ps[:, 0:cw])

        # ---- phase 2: attention, head pairs ----
        # Heads 2j (partitions 0:64) and 2j+1 (64:128) of qT/kT chunk j are
        # processed together: their K=64 scores matmuls are emitted adjacently
        # into different PE row groups, which the hardware runs concurrently.
        # Bias tiles are DMA'd one pair ahead to hide the load latency.
        ba_tiles = {}

        def prefetch_bias(h):
            if h >= NH:
                return
            tiles = []
            for half in range(2):
                ba = biasp.tile([128, 4, N], FP8, tag="biasa", name="ba")
                nc.sync.dma_start(
                    ba[:],
                    biasa_d[h, 512 * half : 512 * (half + 1), :].rearrange(
                        "(t p) m -> p t m", p=128
                    ),
                )
                tiles.append(ba)
            ba_tiles[h] = tiles

        attn_wrs = {}

        def emit_A_pair(j):
            ha, hb = 2 * j, 2 * j + 1
            mark(f"p{j}_A")
            attn_wrs[ha] = []
            attn_wrs[hb] = []
            for half in range(2):
                for nt4 in range(4):
                    nt = 4 * half + nt4
                    pss = {}
                    for h in (ha, hb):
                        pss[h] = psmm.tile([128, N], F32, tag="mm", name="ps_sa")
                    for mh in range(2):
                        for h in (ha, hb):
                            po = (h % 2) * 64
                            nc.tensor.matmul(
                                pss[h][:, 512 * mh : 512 * (mh + 1)],
                                qT[po : po + 64, j, 128 * nt : 128 * (nt + 1)],
                                kT[po : po + 64, j, 512 * mh : 512 * (mh + 1)],
                                start=True,
                                stop=False,
                            )
                        for h in (ha, hb):
                            nc.tensor.matmul(
                                pss[h][:, 512 * mh : 512 * (mh + 1)],
                                ident8[:],
                                ba_tiles[h][half][:, nt4, 512 * mh : 512 * (mh + 1)],
                                start=False,
                                stop=True,
                            )
                    for h in (ha, hb):
                        ex = expap.tile([128, N], BF16, tag="expA", name="ex")
                        ssum = stats.tile([128, 1], F32, tag="ssum", name="ssum")
                        nc.scalar.activation(ex[:], pss[h][:], Exp, accum_out=ssum[:])
                        recip = stats.tile([128, 1], F32, tag="recip", name="recip")
                        nc.vector.reciprocal(recip[:], ssum[:])
                        ast = stage.tile([128, N], BF16, tag="attnstage", name="ast")
                        nc.vector.tensor_scalar_mul(ast[:], ex[:], recip[:])
                        wr = nc.sync.dma_start(
                            attn_d[h, 128 * nt : 128 * (nt + 1), :], ast[:]
                        )
                        attn_wrs[h].append(wr)

        def emit_B_pair(j):
            # One transpose read per (pair, m-block): rows (h*N + n) for both
            # heads -> attnT [m, 2N]. The two heads' U matmuls share PSUM
            # banks via disjoint column groups (0:64 / 64:128).
            ha, hb = 2 * j, 2 * j + 1
            mark(f"p{j}_B")
            psu_t = []
            for nh in range(2):
                psu_t.append(psu.tile([128, 512], F32, tag="u", name="psu_t"))
            for mt in range(NT):
                at = exptp.tile([128, 2 * N], BF16, tag="attnT", name="at")
                rd = nc.sync.dma_start(
                    at[:],
                    attn_d[ha : hb + 1, :, 128 * mt : 128 * (mt + 1)].rearrange(
                        "h n m -> (h n) m"
                    ),
                    transpose=True,
                )
                for wr in attn_wrs[ha] + attn_wrs[hb]:
                    add_dep_helper(rd.ins, wr.ins, reason="attn HBM round-trip RAW")
                for nh in range(2):
                    for idx, h in enumerate((ha, hb)):
                        po = (h % 2) * 64
                        nc.tensor.matmul(
                            psu_t[nh][po : po + 64, :],
                            vX[:, mt, 64 * h : 64 * (h + 1)],
                            at[:, 1024 * idx + 512 * nh : 1024 * idx + 512 * (nh + 1)],
                            start=(mt == 0),
                            stop=(mt == NT - 1),
                        )
            mark(f"p{j}_tail")
            for nh in range(2):
                nc.vector.tensor_copy(
                    outT[:, j, 512 * nh : 512 * (nh + 1)], psu_t[nh][:]
                )

        for h0 in range(4):
            prefetch_bias(h0)
        for j in range(NH // 2):
            emit_qk_chunk(j)
            emit_A_pair(j)
            prefetch_bias(2 * j + 4)
            prefetch_bias(2 * j + 5)
        emit_v(0, 128)
        for j in range(NH // 2):
            emit_B_pair(j)
            if j + 1 < NH // 2:
                emit_v(128 * (j + 1), 128)

        mark("phase3")
        # ---- phase 3: output projection ----
        for nt in range(NT):
            ysb = stage.tile([128, C], F32, tag="y", name="ysb", bufs=2)
            for c0, cw in ((0, 512), (512, 256)):
                ps = psmm.tile([128, 512], F32, tag="mm", name="ps_y")
                for ct in range(CT):
                    nc.tensor.matmul(
                        ps[:, 0:cw],
                        outT[:, ct, 128 * nt : 128 * (nt + 1)],
                        w_sb["wp"][:, ct, c0 : c0 + cw],
                        start=(ct == 0),
                        stop=False,
                    )
                nc.tensor.matmul(
                    ps[:, 0:cw],
                    identb[:],
                    pb_sb[:, c0 : c0 + cw],
                    start=False,
                    stop=True,
                )
                nc.vector.tensor_copy(ysb[:, c0 : c0 + cw], ps[:, 0:cw])
            nc.sync.dma_start(out_d[128 * nt : 128 * (nt + 1), :], ysb[:])

    return nc


_GRAPH_CACHE = {}


def _get_graph():
    if "nc" not in _GRAPH_CACHE:
        nc = build_graph()
        nc.compile()
        _GRAPH_CACHE["nc"] = nc
    return _GRAPH_CACHE["nc"]


def _prep_in_maps(x, Wq, Wk, Wv, Wproj, proj_b, attn_bias, head_bias):
    f = lambda a: np.asarray(a, dtype=np.float32)
    x, Wq, Wk, Wv, Wproj = f(x), f(Wq), f(Wk), f(Wv), f(Wproj)
    proj_b, attn_bias, head_bias = f(proj_b), f(attn_bias), f(head_bias)

    scale = D ** -0.5
    wq_b = (Wq * scale).astype(NP_BF16)
    wk_b = Wk.astype(NP_BF16)
    wv_b = Wv.astype(NP_BF16)
    wp_b = Wproj.astype(NP_BF16)
    bias = attn_bias[None, :, :] + head_bias  # [NH, N, N] f32
    biasa = bias.astype(NP_FP8)
    pb_rep = np.ascontiguousarray(
        np.broadcast_to(proj_b[None, :], (128, C))
    ).astype(NP_BF16)

    in_maps = []
    for b in range(B):
        xt = np.ascontiguousarray(x[b].T).astype(NP_BF16)
        in_maps.append(
            dict(xt=xt, wq=wq_b, wk=wk_b, wv=wv_b, wp=wp_b, pb=pb_rep, biasa=biasa)
        )
    return in_maps


def run(inputs: dict, trace: bool = False, **kw):
    """Build+run on 8 cores; returns (out, attn, BassKernelResults)."""
    in_maps = _prep_in_maps(**inputs)
    nc = _get_graph()
    res = run_bass_kernel_spmd(nc, in_maps, list(range(B)), trace=trace, **kw)
    out = np.stack([r["out"] for r in res.results]).astype(np.float32)
    attn = np.stack([r["attn"] for r in res.results]).astype(np.float32)
    return out, attn, res


def kernel(**inputs):
    out, attn, _ = run(inputs, trace=False)
    return out, attn


# revision 66
# speedup vs baseline: 1.0978x; 1.0013x over previous
"""Self-contained Trainium2 Bass kernel for nn_Attention (B=8, N=1024, C=768, NH=12).

Sharding: pure data-parallel over batch — core b computes batch element b
(projections, 12-head biased softmax attention, attn output, out projection).

Device algorithm per core (matmul inputs bf16, fp32 PSUM accumulate):
  phase 1: qT[e,n], kT[e,n] (e = head*64+d on partitions) and v[m,dd] from
           host-pretransposed xT and weights. 1/sqrt(d) folded into Wq on host.
  phase 2 per head h:
    A-side ([n on partitions]): scores = qT.T@kT + bias (bias fp8, added via
      identity-matmul into the same PSUM accumulation), one Exp on ScalarE
      with fused row-sum accum, per-partition reciprocal, normalize on
      VectorE -> attn tile (bf16) -> DMA to HBM.
    B-side: one DMA-transpose read per (pair, m-block) brings attn of BOTH
      heads back column-wise (bf16 X-bar, [2N,128]->[128,2N]); the two heads'
      U^T = v_h.T @ attnT matmuls accumulate into shared PSUM banks via
      disjoint column groups (concurrent on the PE array), evicted pair-wide
      to outT (c on partitions).
  phase 3: y = outT.T @ Wproj + proj_b, DMA out.

  Emission regions: (qk-chunk j + A-pair j) for all pairs first — attn-write
  DMA traffic starts ~40us earlier and fills the post-load DMA window — then
  the v projection just-in-time per pair between the B sections, whose
  transpose-read streams overlap v/proj PE work.

attn is produced in bf16 on device and upcast to f32 on the host.
proj_b is added on the PE (identity-matmul, bf16) and y evicted on ScalarE.

Cost-model (TimelineSim) exec: ~272 us/core; measured rel_err ~2.8e-3.
"""

from contextlib import ExitStack

import numpy as np
import ml_dtypes

import concourse.mybir as mybir
import concourse.tile as tile
from concourse import bacc
from concourse.bass_utils import run_bass_kernel_spmd
from concourse.masks import make_identity
from concourse.tile_rust import add_dep_helper

BF16 = mybir.dt.bfloat16
FP8 = mybir.dt.float8e4
F32 = mybir.dt.float32
NP_BF16 = ml_dtypes.bfloat16
NP_FP8 = ml_dtypes.float8_e4m3

B, N, C, NH = 8, 1024, 768, 12
D = C // NH  # 64
CT = C // 128  # 6 contraction tiles
NT = N // 128  # 8 sequence tiles
Exp = mybir.ActivationFunctionType.Exp

MARKS = []


def build_graph():
    MARKS.clear()
    nc = bacc.Bacc("TRN2", target_bir_lowering=False, debug=False)
    mark = lambda label: MARKS.append((label, nc.next_id()))

    xt_d = nc.declare_dram_parameter("xt", [C, N], BF16, False).ap()
    wq_d = nc.declare_dram_parameter("wq", [C, C], BF16, False).ap()
    wk_d = nc.declare_dram_parameter("wk", [C, C], BF16, False).ap()
    wv_d = nc.declare_dram_parameter("wv", [C, C], BF16, False).ap()
    wp_d = nc.declare_dram_parameter("wp", [C, C], BF16, False).ap()
    pb_d = nc.declare_dram_parameter("pb", [128, C], BF16, False).ap()
    biasa_d = nc.declare_dram_parameter("biasa", [NH, N, N], FP8, False).ap()
    out_d = nc.declare_dram_parameter("out", [N, C], F32, True).ap()
    attn_d = nc.declare_dram_parameter("attn", [NH, N, N], BF16, True).ap()

    with ExitStack() as ctx:
        tc = ctx.enter_context(tile.TileContext(nc))
        const = ctx.enter_context(tc.tile_pool(name="const", bufs=1))
        persist = ctx.enter_context(tc.tile_pool(name="persist", bufs=1))
        biasp = ctx.enter_context(tc.tile_pool(name="biasp", bufs=8))
        expap = ctx.enter_context(tc.tile_pool(name="expap", bufs=7))
        exptp = ctx.enter_context(tc.tile_pool(name="exptp", bufs=5))
        stage = ctx.enter_context(tc.tile_pool(name="stage", bufs=10))
        stats = ctx.enter_context(tc.tile_pool(name="stats", bufs=4))
        psmm = ctx.enter_context(tc.tile_pool(name="psmm", bufs=3, space="PSUM"))
        psu = ctx.enter_context(tc.tile_pool(name="psu", bufs=2, space="PSUM"))

        ident8 = const.tile([128, 128], FP8, name="ident8")
        make_identity(nc, ident8[:])
        identb = const.tile([128, 128], BF16, name="identb")
        make_identity(nc, identb[:])

        xt_sb = persist.tile([128, CT, N], BF16, name="xt_sb")
        xt_r = xt_d.rearrange("(t p) n -> p t n", p=128)
        for ct in range(CT):
            nc.sync.dma_start(xt_sb[:, ct, :], xt_r[:, ct, :])
        w_sb = {}
        for nm, d_ap in (("wq", wq_d), ("wk", wk_d), ("wv", wv_d), ("wp", wp_d)):
            w_sb[nm] = persist.tile([128, CT, C], BF16, name=f"{nm}_sb")
            nc.sync.dma_start(w_sb[nm][:], d_ap.rearrange("(t p) e -> p t e", p=128))
        pb_sb = persist.tile([128, C], BF16, name="pb_sb")
        nc.sync.dma_start(pb_sb[:], pb_d)

        qT = persist.tile([128, CT, N], BF16, name="qT")
        kT = persist.tile([128, CT, N], BF16, name="kT")
        vX = persist.tile([128, NT, C], BF16, name="vX")
        outT = persist.tile([128, CT, N], BF16, name="outT")

        mark("phase1")

        # ---- projections, emitted per head-pair chunk (see driver) ----
        def emit_qk_chunk(e):
            for wname, dst in (("wq", qT), ("wk", kT)):
                for nh2 in range(2):
                    ps = psmm.tile([128, 512], F32, tag="mm", name="ps_qk")
                    for ct in range(CT):
                        nc.tensor.matmul(
                            ps[:],
                            w_sb[wname][:, ct, 128 * e : 128 * (e + 1)],
                            xt_sb[:, ct, 512 * nh2 : 512 * (nh2 + 1)],
                            start=(ct == 0),
                            stop=(ct == CT - 1),
                        )
                    nc.vector.tensor_copy(
                        dst[:, e, 512 * nh2 : 512 * (nh2 + 1)], ps[:]
                    )

        def emit_v(c0, cw):
            mark(f"v{c0}")
            if True:
                for mt in range(NT):
                    ps = psmm.tile([128, 512], F32, tag="mm", name="ps_v")
                    for ct in range(CT):
                        nc.tensor.matmul(
                            ps[:, 0:cw],
                            xt_sb[:, ct, 128 * mt : 128 * (mt + 1)],
                            w_sb["wv"][:, ct, c0 : c0 + cw],
                            start=(ct == 0),
                            stop=(ct == CT - 1),
                        )
                    nc.vector.tensor_copy(vX[:, mt, c0 : c0 + cw], ps[:, 0:cw])

        # ---- phase 2: attention, head pairs ----
        # Heads 2j (partitions 0:64) and 2j+1 (64:128) of qT/kT chunk j are
        # processed together: their K=64 scores matmuls are emitted adjacently
        # into different PE row groups, which the hardware runs concurrently.
        # Bias tiles are DMA'd one pair ahead to hide the load latency.
        ba_tiles = {}

        def prefetch_bias(h):
            if h >= NH:
                return
            tiles = []
            for half in range(2):
                ba = biasp.tile([128, 4, N], FP8, tag="biasa", name="ba")
                nc.sync.dma_start(
                    ba[:],
                    biasa_d[h, 512 * half : 512 * (half + 1), :].rearrange(
                        "(t p) m -> p t m", p=128
                    ),
                )
                tiles.append(ba)
            ba_tiles[h] = tiles

        attn_wrs = {}

        def emit_A_pair(j):
            ha, hb = 2 * j, 2 * j + 1
            mark(f"p{j}_A")
            attn_wrs[ha] = []
            attn_wrs[hb] = []
            for half in range(2):
                for nt4 in range(4):
                    nt = 4 * half + nt4
                    pss = {}
                    for h in (ha, hb):
                        pss[h] = psmm.tile([128, N], F32, tag="mm", name="ps_sa")
                    for mh in range(2):
                        for h in (ha, hb):
                            po = (h % 2) * 64
                            nc.tensor.matmul(
                                pss[h][:, 512 * mh : 512 * (mh + 1)],
                                qT[po : po + 64, j, 128 * nt : 128 * (nt + 1)],
                                kT[po : po + 64, j, 512 * mh : 512 * (mh + 1)],
                                start=True,
                                stop=False,
                            )
                        for h in (ha, hb):
                            nc.tensor.matmul(
                                pss[h][:, 512 * mh : 512 * (mh + 1)],
                                ident8[:],
                                ba_tiles[h][half][:, nt4, 512 * mh : 512 * (mh + 1)],
                                start=False,
                                stop=True,
                            )
                    for h in (ha, hb):
                        ex = expap.tile([128, N], BF16, tag="expA", name="ex")
                        ssum = stats.tile([128, 1], F32, tag="ssum", name="ssum")
                        nc.scalar.activation(ex[:], pss[h][:], Exp, accum_out=ssum[:])
                        recip = stats.tile([128, 1], F32, tag="recip", name="recip")
                        nc.vector.reciprocal(recip[:], ssum[:])
                        ast = stage.tile([128, N], BF16, tag="attnstage", name="ast")
                        nc.vector.tensor_scalar_mul(ast[:], ex[:], recip[:])
                        wr = nc.sync.dma_start(
                            attn_d[h, 128 * nt : 128 * (nt + 1), :], ast[:]
                        )
                        attn_wrs[h].append(wr)

        def emit_B_pair(j):
            # One transpose read per (pair, m-block): rows (h*N + n) for both
            # heads -> attnT [m, 2N]. The two heads' U matmuls share PSUM
            # banks via disjoint column groups (0:64 / 64:128).
            ha, hb = 2 * j, 2 * j + 1
            mark(f"p{j}_B")
            psu_t = []
            for nh in range(2):
                psu_t.append(psu.tile([128, 512], F32, tag="u", name="psu_t"))
            for mt in range(NT):
                at = exptp.tile([128, 2 * N], BF16, tag="attnT", name="at")
                rd = nc.sync.dma_start(
                    at[:],
                    attn_d[ha : hb + 1, :, 128 * mt : 128 * (mt + 1)].rearrange(
                        "h n m -> (h n) m"
                    ),
                    transpose=True,
                )
                for wr in attn_wrs[ha] + attn_wrs[hb]:
                    add_dep_helper(rd.ins, wr.ins, reason="attn HBM round-trip RAW")
                for nh in range(2):
                    for idx, h in enumerate((ha, hb)):
                        po = (h % 2) * 64
                        nc.tensor.matmul(
                            psu_t[nh][po : po + 64, :],
                            vX[:, mt, 64 * h : 64 * (h + 1)],
                            at[:, 1024 * idx + 512 * nh : 1024 * idx + 512 * (nh + 1)],
                            start=(mt == 0),
                            stop=(mt == NT - 1),
                        )
            mark(f"p{j}_tail")
            for nh in range(2):
                nc.vector.tensor_copy(
                    outT[:, j, 512 * nh : 512 * (nh + 1)], psu_t[nh][:]
                )

        for h0 in range(4):
            prefetch_bias(h0)
        for j in range(NH // 2):
            emit_qk_chunk(j)
            emit_A_pair(j)
            prefetch_bias(2 * j + 4)
            prefetch_bias(2 * j + 5)
        emit_v(0, 128)
        for j in range(NH // 2):
            emit_B_pair(j)
            if j + 1 < NH // 2:
                emit_v(128 * (j + 1), 128)

        mark("phase3")
        # ---- phase 3: output projection ----
        for nt in range(NT):
            ysb = stage.tile([128, C], F32, tag="y", name="ysb", bufs=3)
            for c0, cw in ((0, 512), (512, 256)):
                ps = psmm.tile([128, 512], F32, tag="mm", name="ps_y")
                for ct in range(CT):
                    nc.tensor.matmul(
                        ps[:, 0:cw],
                        outT[:, ct, 128 * nt : 128 * (nt + 1)],
                        w_sb["wp"][:, ct, c0 : c0 + cw],
                        start=(ct == 0),
                        stop=False,
                    )
                nc.tensor.matmul(
                    ps[:, 0:cw],
                    identb[:],
                    pb_sb[:, c0 : c0 + cw],
                    start=False,
                    stop=True,
                )
                nc.vector.tensor_copy(ysb[:, c0 : c0 + cw], ps[:, 0:cw])
            nc.sync.dma_start(out_d[128 * nt : 128 * (nt + 1), :], ysb[:])

    return nc


_GRAPH_CACHE = {}


def _get_graph():
    if "nc" not in _GRAPH_CACHE:
        nc = build_graph()
        nc.compile()
        _GRAPH_CACHE["nc"] = nc
    return _GRAPH_CACHE["nc"]


def _prep_in_maps(x, Wq, Wk, Wv, Wproj, proj_b, attn_bias, head_bias):
    f = lambda a: np.asarray(a, dtype=np.float32)
    x, Wq, Wk, Wv, Wproj = f(x), f(Wq), f(Wk), f(Wv), f(Wproj)
    proj_b, attn_bias, head_bias = f(proj_b), f(attn_bias), f(head_bias)

    scale = D ** -0.5
    wq_b = (Wq * scale).astype(NP_BF16)
    wk_b = Wk.astype(NP_BF16)
    wv_b = Wv.astype(NP_BF16)
    wp_b = Wproj.astype(NP_BF16)
    bias = attn_bias[None, :, :] + head_bias  # [NH, N, N] f32
    biasa = bias.astype(NP_FP8)
    pb_rep = np.ascontiguousarray(
        np.broadcast_to(proj_b[None, :], (128, C))
    ).astype(NP_BF16)

    in_maps = []
    for b in range(B):
        xt = np.ascontiguousarray(x[b].T).astype(NP_BF16)
        in_maps.append(
            dict(xt=xt, wq=wq_b, wk=wk_b, wv=wv_b, wp=wp_b, pb=pb_rep, biasa=biasa)
        )
    return in_maps


def run(inputs: dict, trace: bool = False, **kw):
    """Build+run on 8 cores; returns (out, attn, BassKernelResults)."""
    in_maps = _prep_in_maps(**inputs)
    nc = _get_graph()
    res = run_bass_kernel_spmd(nc, in_maps, list(range(B)), trace=trace, **kw)
    out = np.stack([r["out"] for r in res.results]).astype(np.float32)
    attn = np.stack([r["attn"] for r in res.results]).astype(np.float32)
    return out, attn, res


def kernel(**inputs):
    out, attn, _ = run(inputs, trace=False)
    return out, attn


# revision 71
# speedup vs baseline: 1.0978x; 1.0001x over previous
"""Self-contained Trainium2 Bass kernel for nn_Attention (B=8, N=1024, C=768, NH=12).

Sharding: pure data-parallel over batch — core b computes batch element b
(projections, 12-head biased softmax attention, attn output, out projection).

Device algorithm per core (matmul inputs bf16, fp32 PSUM accumulate):
  phase 1: qT[e,n], kT[e,n] (e = head*64+d on partitions) and v[m,dd] from
           host-pretransposed xT and weights. 1/sqrt(d) folded into Wq on host.
  phase 2 per head h:
    A-side ([n on partitions]): scores = qT.T@kT + bias (bias fp8, added via
      identity-matmul into the same PSUM accumulation), one Exp on ScalarE
      with fused row-sum accum, per-partition reciprocal, normalize on
      VectorE -> attn tile (bf16) -> DMA to HBM.
    B-side: one DMA-transpose read per (pair, m-block) brings attn of BOTH
      heads back column-wise (bf16 X-bar, [2N,128]->[128,2N]); the two heads'
      U^T = v_h.T @ attnT matmuls accumulate into shared PSUM banks via
      disjoint column groups (concurrent on the PE array), evicted pair-wide
      to outT (c on partitions).
  phase 3: y = outT.T @ Wproj + proj_b, DMA out.

  Emission regions: (qk-chunk j + A-pair j) for all pairs first — attn-write
  DMA traffic starts ~40us earlier and fills the post-load DMA window — then
  the v projection just-in-time per pair between the B sections, whose
  transpose-read streams overlap v/proj PE work.

attn is produced in bf16 on device and upcast to f32 on the host.
proj_b is added on the PE (identity-matmul, bf16) and y evicted on ScalarE.

Cost-model (TimelineSim) exec: ~272 us/core; measured rel_err ~2.8e-3.
"""

from contextlib import ExitStack

import numpy as np
import ml_dtypes

import concourse.mybir as mybir
import concourse.tile as tile
from concourse import bacc
from concourse.bass_utils import run_bass_kernel_spmd
from concourse.masks import make_identity
from concourse.tile_rust import add_dep_helper

BF16 = mybir.dt.bfloat16
FP8 = mybir.dt.float8e4
F32 = mybir.dt.float32
NP_BF16 = ml_dtypes.bfloat16
NP_FP8 = ml_dtypes.float8_e4m3

B, N, C, NH = 8, 1024, 768, 12
D = C // NH  # 64
CT = C // 128  # 6 contraction tiles
NT = N // 128  # 8 sequence tiles
Exp = mybir.ActivationFunctionType.Exp

MARKS = []


def build_graph():
    MARKS.clear()
    nc = bacc.Bacc("TRN2", target_bir_lowering=False, debug=False)
    mark = lambda label: MARKS.append((label, nc.next_id()))

    xt_d = nc.declare_dram_parameter("xt", [C, N], BF16, False).ap()
    wq_d = nc.declare_dram_parameter("wq", [C, C], BF16, False).ap()
    wk_d = nc.declare_dram_parameter("wk", [C, C], BF16, False).ap()
    wv_d = nc.declare_dram_parameter("wv", [C, C], BF16, False).ap()
    wp_d = nc.declare_dram_parameter("wp", [C, C], BF16, False).ap()
    pb_d = nc.declare_dram_parameter("pb", [128, C], BF16, False).ap()
    biasa_d = nc.declare_dram_parameter("biasa", [NH, N, N], FP8, False).ap()
    out_d = nc.declare_dram_parameter("out", [N, C], F32, True).ap()
    attn_d = nc.declare_dram_parameter("attn", [NH, N, N], BF16, True).ap()

    with ExitStack() as ctx:
        tc = ctx.enter_context(tile.TileContext(nc))
        const = ctx.enter_context(tc.tile_pool(name="const", bufs=1))
        persist = ctx.enter_context(tc.tile_pool(name="persist", bufs=1))
        biasp = ctx.enter_context(tc.tile_pool(name="biasp", bufs=8))
        expap = ctx.enter_context(tc.tile_pool(name="expap", bufs=7))
        exptp = ctx.enter_context(tc.tile_pool(name="exptp", bufs=5))
        stage = ctx.enter_context(tc.tile_pool(name="stage", bufs=10))
        stats = ctx.enter_context(tc.tile_pool(name="stats", bufs=4))
        psmm = ctx.enter_context(tc.tile_pool(name="psmm", bufs=3, space="PSUM"))
        psu = ctx.enter_context(tc.tile_pool(name="psu", bufs=2, space="PSUM"))

        ident8 = const.tile([128, 128], FP8, name="ident8")
        make_identity(nc, ident8[:])
        identb = const.tile([128, 128], BF16, name="identb")
        make_identity(nc, identb[:])

        xt_sb = persist.tile([128, CT, N], BF16, name="xt_sb")
        xt_r = xt_d.rearrange("(t p) n -> p t n", p=128)
        for ct in range(CT):
            nc.sync.dma_start(xt_sb[:, ct, :], xt_r[:, ct, :])
        w_sb = {}
        for nm, d_ap in (("wq", wq_d), ("wk", wk_d), ("wv", wv_d), ("wp", wp_d)):
            w_sb[nm] = persist.tile([128, CT, C], BF16, name=f"{nm}_sb")
            if nm in ("wq", "wk"):
                nc.sync.dma_start(
                    w_sb[nm][:], d_ap.rearrange("(t p) e -> p t e", p=128)
                )
        pb_sb = persist.tile([128, C], BF16, name="pb_sb")

        def load_late_weights():
            # wv/wp/pb are not needed until the B region / projection — load
            # them after the A region so the ramp DMA window goes to
            # xt/wq/wk/bias and the first attn writes.
            for nm, d_ap in (("wv", wv_d), ("wp", wp_d)):
                nc.sync.dma_start(
                    w_sb[nm][:], d_ap.rearrange("(t p) e -> p t e", p=128)
                )
            nc.sync.dma_start(pb_sb[:], pb_d)

        qT = persist.tile([128, CT, N], BF16, name="qT")
        kT = persist.tile([128, CT, N], BF16, name="kT")
        vX = persist.tile([128, NT, C], BF16, name="vX")
        outT = persist.tile([128, CT, N], BF16, name="outT")

        mark("phase1")

        # ---- projections, emitted per head-pair chunk (see driver) ----
        def emit_qk_chunk(e):
            for wname, dst in (("wq", qT), ("wk", kT)):
                for nh2 in range(2):
                    ps = psmm.tile([128, 512], F32, tag="mm", name="ps_qk")
                    for ct in range(CT):
                        nc.tensor.matmul(
                            ps[:],
                            w_sb[wname][:, ct, 128 * e : 128 * (e + 1)],
                            xt_sb[:, ct, 512 * nh2 : 512 * (nh2 + 1)],
                            start=(ct == 0),
                            stop=(ct == CT - 1),
                        )
                    nc.vector.tensor_copy(
                        dst[:, e, 512 * nh2 : 512 * (nh2 + 1)], ps[:]
                    )

        def emit_v(c0, cw):
            mark(f"v{c0}")
            if True:
                for mt in range(NT):
                    ps = psmm.tile([128, 512], F32, tag="mm", name="ps_v")
                    for ct in range(CT):
                        nc.tensor.matmul(
                            ps[:, 0:cw],
                            xt_sb[:, ct, 128 * mt : 128 * (mt + 1)],
                            w_sb["wv"][:, ct, c0 : c0 + cw],
                            start=(ct == 0),
                            stop=(ct == CT - 1),
                        )
                    nc.vector.tensor_copy(vX[:, mt, c0 : c0 + cw], ps[:, 0:cw])

        # ---- phase 2: attention, head pairs ----
        # Heads 2j (partitions 0:64) and 2j+1 (64:128) of qT/kT chunk j are
        # processed together: their K=64 scores matmuls are emitted adjacently
        # into different PE row groups, which the hardware runs concurrently.
        # Bias tiles are DMA'd one pair ahead to hide the load latency.
        ba_tiles = {}

        def prefetch_bias(h):
            if h >= NH:
                return
            tiles = []
            for half in range(2):
                ba = biasp.tile([128, 4, N], FP8, tag="biasa", name="ba")
                nc.sync.dma_start(
                    ba[:],
                    biasa_d[h, 512 * half : 512 * (half + 1), :].rearrange(
                        "(t p) m -> p t m", p=128
                    ),
                )
                tiles.append(ba)
            ba_tiles[h] = tiles

        attn_wrs = {}

        def emit_A_pair(j):
            ha, hb = 2 * j, 2 * j + 1
            mark(f"p{j}_A")
            attn_wrs[ha] = []
            attn_wrs[hb] = []
            for half in range(2):
                for nt4 in range(4):
                    nt = 4 * half + nt4
                    pss = {}
                    for h in (ha, hb):
                        pss[h] = psmm.tile([128, N], F32, tag="mm", name="ps_sa")
                    for mh in range(2):
                        for h in (ha, hb):
                            po = (h % 2) * 64
                            nc.tensor.matmul(
                                pss[h][:, 512 * mh : 512 * (mh + 1)],
                                qT[po : po + 64, j, 128 * nt : 128 * (nt + 1)],
                                kT[po : po + 64, j, 512 * mh : 512 * (mh + 1)],
                                start=True,
                                stop=False,
                            )
                        for h in (ha, hb):
                            nc.tensor.matmul(
                                pss[h][:, 512 * mh : 512 * (mh + 1)],
                                ident8[:],
                                ba_tiles[h][half][:, nt4, 512 * mh : 512 * (mh + 1)],
                                start=False,
                                stop=True,
                            )
                    for h in (ha, hb):
                        ex = expap.tile([128, N], BF16, tag="expA", name="ex")
                        ssum = stats.tile([128, 1], F32, tag="ssum", name="ssum")
                        nc.scalar.activation(ex[:], pss[h][:], Exp, accum_out=ssum[:])
                        recip = stats.tile([128, 1], F32, tag="recip", name="recip")
                        nc.vector.reciprocal(recip[:], ssum[:])
                        ast = stage.tile([128, N], BF16, tag="attnstage", name="ast")
                        nc.vector.tensor_scalar_mul(ast[:], ex[:], recip[:])
                        wr = nc.sync.dma_start(
                            attn_d[h, 128 * nt : 128 * (nt + 1), :], ast[:]
                        )
                        attn_wrs[h].append(wr)

        def emit_B_pair(j):
            # One transpose read per (pair, m-block): rows (h*N + n) for both
            # heads -> attnT [m, 2N]. The two heads' U matmuls share PSUM
            # banks via disjoint column groups (0:64 / 64:128).
            ha, hb = 2 * j, 2 * j + 1
            mark(f"p{j}_B")
            psu_t = []
            for nh in range(2):
                psu_t.append(psu.tile([128, 512], F32, tag="u", name="psu_t"))
            for mt in range(NT):
                at = exptp.tile([128, 2 * N], BF16, tag="attnT", name="at")
                rd = nc.sync.dma_start(
                    at[:],
                    attn_d[ha : hb + 1, :, 128 * mt : 128 * (mt + 1)].rearrange(
                        "h n m -> (h n) m"
                    ),
                    transpose=True,
                )
                for wr in attn_wrs[ha] + attn_wrs[hb]:
                    add_dep_helper(rd.ins, wr.ins, reason="attn HBM round-trip RAW")
                for nh in range(2):
                    for idx, h in enumerate((ha, hb)):
                        po = (h % 2) * 64
                        nc.tensor.matmul(
                            psu_t[nh][po : po + 64, :],
                            vX[:, mt, 64 * h : 64 * (h + 1)],
                            at[:, 1024 * idx + 512 * nh : 1024 * idx + 512 * (nh + 1)],
                            start=(mt == 0),
                            stop=(mt == NT - 1),
                        )
            mark(f"p{j}_tail")
            for nh in range(2):
                nc.vector.tensor_copy(
                    outT[:, j, 512 * nh : 512 * (nh + 1)], psu_t[nh][:]
                )

        for h0 in range(4):
            prefetch_bias(h0)
        for j in range(NH // 2):
            emit_qk_chunk(j)
            emit_A_pair(j)
            prefetch_bias(2 * j + 4)
            prefetch_bias(2 * j + 5)
        load_late_weights()
        emit_v(0, 128)
        for j in range(NH // 2):
            emit_B_pair(j)
            if j + 1 < NH // 2:
                emit_v(128 * (j + 1), 128)

        mark("phase3")
        # ---- phase 3: output projection ----
        for nt in range(NT):
            ysb = stage.tile([128, C], F32, tag="y", name="ysb", bufs=3)
            for c0, cw in ((0, 512), (512, 256)):
                ps = psmm.tile([128, 512], F32, tag="mm", name="ps_y")
                for ct in range(CT):
                    nc.tensor.matmul(
                        ps[:, 0:cw],
                        outT[:, ct, 128 * nt : 128 * (nt + 1)],
                        w_sb["wp"][:, ct, c0 : c0 + cw],
                        start=(ct == 0),
                        stop=False,
                    )
                nc.tensor.matmul(
                    ps[:, 0:cw],
                    identb[:],
                    pb_sb[:, c0 : c0 + cw],
                    start=False,
                    stop=True,
                )
                nc.vector.tensor_copy(ysb[:, c0 : c0 + cw], ps[:, 0:cw])
            nc.sync.dma_start(out_d[128 * nt : 128 * (nt + 1), :], ysb[:])

    return nc


_GRAPH_CACHE = {}


def _get_graph():
    if "nc" not in _GRAPH_CACHE:
        nc = build_graph()
        nc.compile()
        _GRAPH_CACHE["nc"] = nc
    return _GRAPH_CACHE["nc"]


def _prep_in_maps(x, Wq, Wk, Wv, Wproj, proj_b, attn_bias, head_bias):
    f = lambda a: np.asarray(a, dtype=np.float32)
    x, Wq, Wk, Wv, Wproj = f(x), f(Wq), f(Wk), f(Wv), f(Wproj)
    proj_b, attn_bias, head_bias = f(proj_b), f(attn_bias), f(head_bias)

    scale = D ** -0.5
    wq_b = (Wq * scale).astype(NP_BF16)
    wk_b = Wk.astype(NP_BF16)
    wv_b = Wv.astype(NP_BF16)
    wp_b = Wproj.astype(NP_BF16)
    bias = attn_bias[None, :, :] + head_bias  # [NH, N, N] f32
    biasa = bias.astype(NP_FP8)
    pb_rep = np.ascontiguousarray(
        np.broadcast_to(proj_b[None, :], (128, C))
    ).astype(NP_BF16)

    in_maps = []
    for b in range(B):
        xt = np.ascontiguousarray(x[b].T).astype(NP_BF16)
        in_maps.append(
            dict(xt=xt, wq=wq_b, wk=wk_b, wv=wv_b, wp=wp_b, pb=pb_rep, biasa=biasa)
        )
    return in_maps


def run(inputs: dict, trace: bool = False, **kw):
    """Build+run on 8 cores; returns (out, attn, BassKernelResults)."""
    in_maps = _prep_in_maps(**inputs)
    nc = _get_graph()
    res = run_bass_kernel_spmd(nc, in_maps, list(range(B)), trace=trace, **kw)
    out = np.stack([r["out"] for r in res.results]).astype(np.float32)
    attn = np.stack([r["attn"] for r in res.results]).astype(np.float32)
    return out, attn, res


def kernel(**inputs):
    out, attn, _ = run(inputs, trace=False)
    return out, attn


# revision 83
# speedup vs baseline: 1.1115x; 1.0125x over previous
"""Self-contained Trainium2 Bass kernel for nn_Attention (B=8, N=1024, C=768, NH=12).

Sharding: pure data-parallel over batch — core b computes batch element b
(projections, 12-head biased softmax attention, attn output, out projection).

Device algorithm per core (matmul inputs bf16, fp32 PSUM accumulate):
  phase 1: qT[e,n], kT[e,n] (e = head*64+d on partitions) and v[m,dd] from
           host-pretransposed xT and weights. 1/sqrt(d) folded into Wq on host.
  phase 2 per head h:
    A-side ([n on partitions]): scores = qT.T@kT + bias (bias fp8, added via
      identity-matmul into the same PSUM accumulation), one Exp on ScalarE
      with fused row-sum accum, per-partition reciprocal, normalize on
      VectorE -> attn tile (bf16) -> DMA to HBM.
    B-side: one DMA-transpose read per (pair, m-block) brings attn of BOTH
      heads back column-wise (bf16 X-bar, [2N,128]->[128,2N]); the two heads'
      U^T = v_h.T @ attnT matmuls accumulate into shared PSUM banks via
      disjoint column groups (concurrent on the PE array), evicted pair-wide
      to outT (c on partitions).
  phase 3: y = outT.T @ Wproj + proj_b, DMA out.

  Emission regions: (qk-chunk j + A-pair j) for all pairs first — attn-write
  DMA traffic starts ~40us earlier and fills the post-load DMA window — then
  the v projection just-in-time per pair between the B sections, whose
  transpose-read streams overlap v/proj PE work.

attn is produced in bf16 on device and upcast to f32 on the host.
proj_b is added on the PE (identity-matmul, bf16) and y evicted on ScalarE.

Cost-model (TimelineSim) exec: ~268 us/core; measured rel_err ~2.8e-3.
"""

from contextlib import ExitStack

import numpy as np
import ml_dtypes

import concourse.mybir as mybir
import concourse.tile as tile
from concourse import bacc
from concourse.bass_utils import run_bass_kernel_spmd
from concourse.masks import make_identity
from concourse.tile_rust import add_dep_helper

BF16 = mybir.dt.bfloat16
FP8 = mybir.dt.float8e4
F32 = mybir.dt.float32
NP_BF16 = ml_dtypes.bfloat16
NP_FP8 = ml_dtypes.float8_e4m3

B, N, C, NH = 8, 1024, 768, 12
D = C // NH  # 64
CT = C // 128  # 6 contraction tiles
NT = N // 128  # 8 sequence tiles
Exp = mybir.ActivationFunctionType.Exp

MARKS = []


def build_graph():
    MARKS.clear()
    nc = bacc.Bacc("TRN2", target_bir_lowering=False, debug=False)
    mark = lambda label: MARKS.append((label, nc.next_id()))

    xt_d = nc.declare_dram_parameter("xt", [C, N], BF16, False).ap()
    wq_d = nc.declare_dram_parameter("wq", [C, C], BF16, False).ap()
    wk_d = nc.declare_dram_parameter("wk", [C, C], BF16, False).ap()
    wv_d = nc.declare_dram_parameter("wv", [C, C], BF16, False).ap()
    wp_d = nc.declare_dram_parameter("wp", [C, C], BF16, False).ap()
    pb_d = nc.declare_dram_parameter("pb", [128, C], BF16, False).ap()
    biasa_d = nc.declare_dram_parameter("biasa", [NH, N, N], FP8, False).ap()
    out_d = nc.declare_dram_parameter("out", [N, C], F32, True).ap()
    attn_d = nc.declare_dram_parameter("attn", [NH, N, N], BF16, True).ap()

    with ExitStack() as ctx:
        tc = ctx.enter_context(tile.TileContext(nc))
        const = ctx.enter_context(tc.tile_pool(name="const", bufs=1))
        persist = ctx.enter_context(tc.tile_pool(name="persist", bufs=1))
        biasp = ctx.enter_context(tc.tile_pool(name="biasp", bufs=8))
        expap = ctx.enter_context(tc.tile_pool(name="expap", bufs=7))
        exptp = ctx.enter_context(tc.tile_pool(name="exptp", bufs=5))
        stage = ctx.enter_context(tc.tile_pool(name="stage", bufs=16))
        stats = ctx.enter_context(tc.tile_pool(name="stats", bufs=4))
        psmm = ctx.enter_context(tc.tile_pool(name="psmm", bufs=3, space="PSUM"))
        psu = ctx.enter_context(tc.tile_pool(name="psu", bufs=2, space="PSUM"))

        ident8 = const.tile([128, 128], FP8, name="ident8")
        make_identity(nc, ident8[:])
        identb = const.tile([128, 128], BF16, name="identb")
        make_identity(nc, identb[:])

        xt_sb = persist.tile([128, CT, N], BF16, name="xt_sb")
        xt_r = xt_d.rearrange("(t p) n -> p t n", p=128)
        for ct in range(CT):
            nc.sync.dma_start(xt_sb[:, ct, :], xt_r[:, ct, :])
        w_sb = {}
        for nm, d_ap in (("wq", wq_d), ("wk", wk_d), ("wv", wv_d), ("wp", wp_d)):
            w_sb[nm] = persist.tile([128, CT, C], BF16, name=f"{nm}_sb")
            if nm in ("wq", "wk"):
                nc.sync.dma_start(
                    w_sb[nm][:], d_ap.rearrange("(t p) e -> p t e", p=128)
                )
        pb_sb = persist.tile([128, C], BF16, name="pb_sb")

        def load_late_weights():
            # wv/wp/pb are not needed until the B region / projection — load
            # them after the A region so the ramp DMA window goes to
            # xt/wq/wk/bias and the first attn writes.
            for nm, d_ap in (("wv", wv_d), ("wp", wp_d)):
                nc.sync.dma_start(
                    w_sb[nm][:], d_ap.rearrange("(t p) e -> p t e", p=128)
                )
            nc.sync.dma_start(pb_sb[:], pb_d)

        qT = persist.tile([128, CT, N], BF16, name="qT")
        kT = persist.tile([128, CT, N], BF16, name="kT")
        vX = persist.tile([128, NT, C], BF16, name="vX")
        outT = persist.tile([128, CT, N], BF16, name="outT")

        mark("phase1")

        # ---- projections, emitted per head-pair chunk (see driver) ----
        def emit_qk_chunk(e):
            for wname, dst in (("wq", qT), ("wk", kT)):
                for nh2 in range(2):
                    ps = psmm.tile([128, 512], F32, tag="mm", name="ps_qk")
                    for ct in range(CT):
                        nc.tensor.matmul(
                            ps[:],
                            w_sb[wname][:, ct, 128 * e : 128 * (e + 1)],
                            xt_sb[:, ct, 512 * nh2 : 512 * (nh2 + 1)],
                            start=(ct == 0),
                            stop=(ct == CT - 1),
                        )
                    nc.vector.tensor_copy(
                        dst[:, e, 512 * nh2 : 512 * (nh2 + 1)], ps[:]
                    )

        def emit_v(c0, cw):
            mark(f"v{c0}")
            if True:
                for mt in range(NT):
                    ps = psmm.tile([128, 512], F32, tag="mm", name="ps_v")
                    for ct in range(CT):
                        nc.tensor.matmul(
                            ps[:, 0:cw],
                            xt_sb[:, ct, 128 * mt : 128 * (mt + 1)],
                            w_sb["wv"][:, ct, c0 : c0 + cw],
                            start=(ct == 0),
                            stop=(ct == CT - 1),
                        )
                    nc.vector.tensor_copy(vX[:, mt, c0 : c0 + cw], ps[:, 0:cw])

        # ---- phase 2: attention, head pairs ----
        # Heads 2j (partitions 0:64) and 2j+1 (64:128) of qT/kT chunk j are
        # processed together: their K=64 scores matmuls are emitted adjacently
        # into different PE row groups, which the hardware runs concurrently.
        # Bias tiles are DMA'd one pair ahead to hide the load latency.
        ba_tiles = {}

        def prefetch_bias(h):
            if h >= NH:
                return
            tiles = []
            for half in range(2):
                ba = biasp.tile([128, 4, N], FP8, tag="biasa", name="ba")
                nc.sync.dma_start(
                    ba[:],
                    biasa_d[h, 512 * half : 512 * (half + 1), :].rearrange(
                        "(t p) m -> p t m", p=128
                    ),
                )
                tiles.append(ba)
            ba_tiles[h] = tiles

        attn_wrs = {}

        def emit_A_pair(j):
            ha, hb = 2 * j, 2 * j + 1
            mark(f"p{j}_A")
            attn_wrs[ha] = []
            attn_wrs[hb] = []
            for half in range(2):
                for nt4 in range(4):
                    nt = 4 * half + nt4
                    pss = {}
                    for h in (ha, hb):
                        pss[h] = psmm.tile([128, N], F32, tag="mm", name="ps_sa")
                    for mh in range(2):
                        for h in (ha, hb):
                            po = (h % 2) * 64
                            nc.tensor.matmul(
                                pss[h][:, 512 * mh : 512 * (mh + 1)],
                                qT[po : po + 64, j, 128 * nt : 128 * (nt + 1)],
                                kT[po : po + 64, j, 512 * mh : 512 * (mh + 1)],
                                start=True,
                                stop=False,
                            )
                        for h in (ha, hb):
                            nc.tensor.matmul(
                                pss[h][:, 512 * mh : 512 * (mh + 1)],
                                ident8[:],
                                ba_tiles[h][half][:, nt4, 512 * mh : 512 * (mh + 1)],
                                start=False,
                                stop=True,
                            )
                    for h in (ha, hb):
                        ex = expap.tile([128, N], BF16, tag="expA", name="ex")
                        ssum = stats.tile([128, 1], F32, tag="ssum", name="ssum")
                        nc.scalar.activation(ex[:], pss[h][:], Exp, accum_out=ssum[:])
                        recip = stats.tile([128, 1], F32, tag="recip", name="recip")
                        nc.vector.reciprocal(recip[:], ssum[:])
                        ast = stage.tile([128, N], BF16, tag="attnstage", name="ast")
                        nc.vector.tensor_scalar_mul(ast[:], ex[:], recip[:])
                        wr = nc.sync.dma_start(
                            attn_d[h, 128 * nt : 128 * (nt + 1), :], ast[:]
                        )
                        attn_wrs[h].append(wr)

        def emit_B_pair(j):
            # One transpose read per (pair, m-block): rows (h*N + n) for both
            # heads -> attnT [m, 2N]. The two heads' U matmuls share PSUM
            # banks via disjoint column groups (0:64 / 64:128).
            ha, hb = 2 * j, 2 * j + 1
            mark(f"p{j}_B")
            psu_t = []
            for nh in range(2):
                psu_t.append(psu.tile([128, 512], F32, tag="u", name="psu_t"))
            for mt in range(NT):
                at = exptp.tile([128, 2 * N], BF16, tag="attnT", name="at")
                rd = nc.sync.dma_start(
                    at[:],
                    attn_d[ha : hb + 1, :, 128 * mt : 128 * (mt + 1)].rearrange(
                        "h n m -> (h n) m"
                    ),
                    transpose=True,
                )
                for wr in attn_wrs[ha] + attn_wrs[hb]:
                    add_dep_helper(rd.ins, wr.ins, reason="attn HBM round-trip RAW")
                for nh in range(2):
                    for idx, h in enumerate((ha, hb)):
                        po = (h % 2) * 64
                        nc.tensor.matmul(
                            psu_t[nh][po : po + 64, :],
                            vX[:, mt, 64 * h : 64 * (h + 1)],
                            at[:, 1024 * idx + 512 * nh : 1024 * idx + 512 * (nh + 1)],
                            start=(mt == 0),
                            stop=(mt == NT - 1),
                        )
            mark(f"p{j}_tail")
            for nh in range(2):
                nc.vector.tensor_copy(
                    outT[:, j, 512 * nh : 512 * (nh + 1)], psu_t[nh][:]
                )

        for h0 in range(4):
            prefetch_bias(h0)
        for j in range(NH // 2):
            emit_qk_chunk(j)
            emit_A_pair(j)
            prefetch_bias(2 * j + 4)
            prefetch_bias(2 * j + 5)
        load_late_weights()
        emit_v(0, 128)
        for j in range(NH // 2):
            emit_B_pair(j)
            if j + 1 < NH // 2:
                emit_v(128 * (j + 1), 128)

        mark("phase3")
        # ---- phase 3: output projection ----
        for nt in range(NT):
            ysb = stage.tile([128, C], F32, tag="y", name="ysb", bufs=3)
            for c0, cw in ((0, 512), (512, 256)):
                ps = psmm.tile([128, 512], F32, tag="mm", name="ps_y")
                for ct in range(CT):
                    nc.tensor.matmul(
                        ps[:, 0:cw],
                        outT[:, ct, 128 * nt : 128 * (nt + 1)],
                        w_sb["wp"][:, ct, c0 : c0 + cw],
                        start=(ct == 0),
                        stop=False,
                    )
                nc.tensor.matmul(
                    ps[:, 0:cw],
                    identb[:],
                    pb_sb[:, c0 : c0 + cw],
                    start=False,
                    stop=True,
                )
                nc.vector.tensor_copy(ysb[:, c0 : c0 + cw], ps[:, 0:cw])
            nc.sync.dma_start(out_d[128 * nt : 128 * (nt + 1), :], ysb[:])

    return nc


_GRAPH_CACHE = {}


def _get_graph():
    if "nc" not in _GRAPH_CACHE:
        nc = build_graph()
        nc.compile()
        _GRAPH_CACHE["nc"] = nc
    return _GRAPH_CACHE["nc"]


def _prep_in_maps(x, Wq, Wk, Wv, Wproj, proj_b, attn_bias, head_bias):
    f = lambda a: np.asarray(a, dtype=np.float32)
    x, Wq, Wk, Wv, Wproj = f(x), f(Wq), f(Wk), f(Wv), f(Wproj)
    proj_b, attn_bias, head_bias = f(proj_b), f(attn_bias), f(head_bias)

    scale = D ** -0.5
    wq_b = (Wq * scale).astype(NP_BF16)
    wk_b = Wk.astype(NP_BF16)
    wv_b = Wv.astype(NP_BF16)
    wp_b = Wproj.astype(NP_BF16)
    bias = attn_bias[None, :, :] + head_bias  # [NH, N, N] f32
    biasa = bias.astype(NP_FP8)
    pb_rep = np.ascontiguousarray(
        np.broadcast_to(proj_b[None, :], (128, C))
    ).astype(NP_BF16)

    in_maps = []
    for b in range(B):
        xt = np.ascontiguousarray(x[b].T).astype(NP_BF16)
        in_maps.append(
            dict(xt=xt, wq=wq_b, wk=wk_b, wv=wv_b, wp=wp_b, pb=pb_rep, biasa=biasa)
        )
    return in_maps


def run(inputs: dict, trace: bool = False, **kw):
    """Build+run on 8 cores; returns (out, attn, BassKernelResults)."""
    in_maps = _prep_in_maps(**inputs)
    nc = _get_graph()
    res = run_bass_kernel_spmd(nc, in_maps, list(range(B)), trace=trace, **kw)
    out = np.stack([r["out"] for r in res.results]).astype(np.float32)
    attn = np.stack([r["attn"] for r in res.results]).astype(np.float32)
    return out, attn, res


def kernel(**inputs):
    out, attn, _ = run(inputs, trace=False)
    return out, attn
